# revision 21
# baseline (speedup 1.0000x reference)
"""CTC greedy decode kernel for Trainium2 (Bass/Tile), 8-core data-parallel.

Problem: log_probs [32, 4096, 1025] f32, input_lengths [32] i64 ->
  preds    [32, 4096] int32  (per-frame argmax)
  keep     [32, 4096] bool   (non-blank & != prev & t < len)
  max_logp [32, 4096] f32    (value at argmax)

Sharding: batch dim across 8 cores (4 utterances each). Per core:
16384 frames x 1025 vocab, viewed as a [128, 128] grid of frame-tiles
with frame f = p*128 + c (p = SBUF partition, c = grid column). Each
utterance owns 32 partition rows and time runs along the FREE dim:
t = (p%32)*128 + c. That makes the CTC "previous frame" a plain
shifted-AP compare (idxf[:, c] vs idxf[:, c-1]) -- no partition-shift
DMA, no DMA-semaphore hop on the critical tail. Only column 0 wraps
(prev of frame (p%32)*128 is (p-1, 127)), staged via one tiny DMA that
overlaps the last column's compute by processing columns in the order
1..127, then 0.

Argmax without a second DVE scan (the two-pass reduce+max_index version
is vector-bound at ~273us; DMA of the 67MB/core input is ~187us):

  1. DVE tensor_tensor_scan (op0=op1=max, data1=data0) computes the
     running prefix-max P_v of each frame in ONE pass. Its last element
     is the frame max m (exact f32, also the max_logp output).
  2. The Activation engine computes Sign(-P_v + m) -- 1 where P_v < m,
     0 where P_v == m -- and its accum_out sums the pass: the count of
     prefix positions strictly below the max IS the argmax index, with
     exact first-occurrence tie-breaking (jnp.argmax semantics) for any
     input, duplicates included.

So DVE does one 1.04ns/elem pass (~150us), ACT one 0.83ns/elem pass
(~160us incl per-inst SBUF access), and the ~188us HBM stream is the
critical path. The last 15 columns load per-column so DVE/ACT drain
their one-group pipeline lag before the stream ends; the post-stream
tail is one column's load-sem+scan+sign plus a couple of [128,1] mask
ops and the final store.
"""

import numpy as np

import concourse.bacc as bacc
import concourse.mybir as mybir
from concourse.tile import TileContext
from concourse.bass_utils import run_bass_kernel_spmd

B, T, V = 32, 4096, 1025
BLANK = 1024
NCORES = 8
BLOC = B // NCORES        # utterances per core
F = BLOC * T              # frames per core
P = 128                   # partitions
NT = F // P               # grid columns (128)
RPU = P // BLOC           # partition rows per utterance (32)
NGRP = 28                 # 4-col groups covering columns 1..112
PERCOL0 = 113             # columns 113..127 load per-column, col 0 last

_CACHE = {}


def _build_program():
    nc = bacc.Bacc(None, target_bir_lowering=False)
    f32 = mybir.dt.float32
    i32 = mybir.dt.int32
    lp = nc.dram_tensor("lp", [F, V], f32, kind="ExternalInput")
    valid = nc.dram_tensor("valid", [P, NT], f32, kind="ExternalInput")
    preds_o = nc.dram_tensor("preds", [P, NT], i32, kind="ExternalOutput")
    keep_o = nc.dram_tensor("keep", [P, NT], i32, kind="ExternalOutput")
    mlp_o = nc.dram_tensor("maxlp", [P, NT], f32, kind="ExternalOutput")

    # frame f = p*128 + c  ->  [p, c, v]; a 4-column tile is a contiguous
    # 16400B run per partition in HBM
    lp_r = lp.rearrange("(p n) v -> p n v", n=NT)
    SIGN = mybir.ActivationFunctionType.Sign

    with TileContext(nc) as tc:
        with (
            tc.tile_pool(name="loads", bufs=4) as loads,
            tc.tile_pool(name="tails", bufs=8) as tails,
            tc.tile_pool(name="pms", bufs=3) as pms,
            tc.tile_pool(name="pmts", bufs=5) as pmts,
            tc.tile_pool(name="sgs", bufs=3) as sgs,
            tc.tile_pool(name="persist", bufs=1) as pp,
        ):
            first = loads.tile([P, 4, V], f32, tag="big")
            nc.sync.dma_start(out=first[:], in_=lp_r[:, 1:5, :])

            idxf = pp.tile([P, NT], f32)     # argmax index (exact int in f32)
            gmax = pp.tile([P, NT], f32)     # frame max (max_logp output)
            validt = pp.tile([P, NT], f32)
            wrap = pp.tile([P, 1], f32)      # prev for column 0
            k1 = pp.tile([P, NT], f32)
            kp = pp.tile([P, NT], f32)
            preds_i = pp.tile([P, NT], i32)
            keep_i = pp.tile([P, NT], i32)

            nc.sync.dma_start(out=validt[:], in_=valid[:])

            def tile_pass(src2d, pm2d, col):
                # one frame-column: prefix-max scan, then Sign+accumulate
                nc.vector.tensor_tensor_scan(
                    out=pm2d, data0=src2d, data1=src2d,
                    initial=-3.0e38,
                    op0=mybir.AluOpType.max, op1=mybir.AluOpType.max,
                )
                sg = sgs.tile([P, V], f32, tag="sg")
                nc.scalar.activation(
                    out=sg[:], in_=pm2d, func=SIGN,
                    bias=pm2d[:, V - 1 : V], scale=-1.0,
                    accum_out=idxf[:, col : col + 1],
                )

            def percol(c):
                bt = tails.tile([P, 1, V], f32, tag="tail")
                nc.sync.dma_start(out=bt[:], in_=lp_r[:, c : c + 1, :])
                pmt = pmts.tile([P, 1, V], f32, tag="pmt")
                tile_pass(bt[:, 0, :], pmt[:, 0, :], c)
                nc.gpsimd.tensor_copy(
                    out=gmax[:, c : c + 1], in_=pmt[:, 0, V - 1 : V]
                )

            # columns 1..112 in 4-column tiles
            for g in range(NGRP):
                c0 = 1 + g * 4
                if g == 0:
                    big = first
                else:
                    big = loads.tile([P, 4, V], f32, tag="big")
                    nc.sync.dma_start(out=big[:], in_=lp_r[:, c0 : c0 + 4, :])
                pm = pms.tile([P, 4, V], f32, tag="pm")
                for i in range(4):
                    tile_pass(big[:, i, :], pm[:, i, :], c0 + i)
                nc.gpsimd.tensor_copy(
                    out=gmax[:, c0 : c0 + 4], in_=pm[:, :, V - 1]
                )

            # columns 113..127 per-column (lets DVE/ACT drain their lag).
            # Column 127 runs six slots early so the wrap staging below has
            # its data (and its DMA-sem latency) retired well before the
            # final column's compute needs it.
            order = list(range(PERCOL0, NT - 1))
            order = order[:9] + [NT - 1] + order[9:]
            for c in order:
                percol(c)

            # column 0 last, split into two half-vocab loads with a chained
            # prefix-max so the final scan work after the stream's last
            # byte is one half, not a full column; its argmax comes from a
            # DVE max_index (exact first-occurrence for a single frame) so
            # the final mask ops don't wait on an ACT->DVE handoff
            HV = 513
            bt0 = tails.tile([P, 1, V], f32, tag="tail")
            nc.sync.dma_start(out=bt0[:, :, 0:HV], in_=lp_r[:, 0:1, 0:HV])
            nc.sync.dma_start(out=bt0[:, :, HV:V], in_=lp_r[:, 0:1, HV:V])

            # stage prev-of-column-0: wrap[p] = idxf[p-1, 127]; rows at
            # utterance starts become the -1 sentinel. Emitted after every
            # load so its ACT_127 sem-wait can't block the SP load stream;
            # col 127 ran early, so the staging retires well before kp0.
            nc.sync.dma_start(
                out=wrap[1:P, :], in_=idxf[0 : P - 1, NT - 1 : NT]
            )
            for u in range(BLOC):
                nc.gpsimd.memset(wrap[u * RPU : u * RPU + 1, :], -1.0)
            pmt0 = pmts.tile([P, 1, V], f32, tag="pmt")
            nc.vector.tensor_tensor_scan(
                out=pmt0[:, 0, 0:HV], data0=bt0[:, 0, 0:HV],
                data1=bt0[:, 0, 0:HV], initial=-3.0e38,
                op0=mybir.AluOpType.max, op1=mybir.AluOpType.max,
            )
            nc.vector.tensor_tensor_scan(
                out=pmt0[:, 0, HV:V], data0=bt0[:, 0, HV:V],
                data1=bt0[:, 0, HV:V], initial=pmt0[:, 0, HV - 1 : HV],
                op0=mybir.AluOpType.max, op1=mybir.AluOpType.max,
            )

            # column 0's index via ACT sign+accum: the ACT engine is idle
            # after column 126, and this frees DVE to run the [1:NT] mask
            # ops concurrently with column 0's sign pass
            sg0 = sgs.tile([P, V], f32, tag="sg")
            nc.scalar.activation(
                out=sg0[:], in_=pmt0[:, 0, :], func=SIGN,
                bias=pmt0[:, 0, V - 1 : V], scale=-1.0,
                accum_out=idxf[:, 0:1],
            )
            nc.gpsimd.tensor_copy(
                out=gmax[:, 0:1], in_=pmt0[:, 0, V - 1 : V]
            )

            # epilogue: full-width masks and exactly three output stores.
            # Fewer, bigger stores beat split early/late stores -- each
            # extra DMA costs ~700ns of serialized HWDGE at the very end,
            # more than the ~1.2us by which column 0 trails the rest.
            nc.sync.dma_start(out=mlp_o[:], in_=gmax[:])
            nc.vector.tensor_copy(out=preds_i[:, 1:NT], in_=idxf[:, 1:NT])
            nc.vector.tensor_scalar(
                out=k1[:, 1:NT], in0=idxf[:, 1:NT],
                scalar1=float(BLANK), scalar2=None,
                op0=mybir.AluOpType.not_equal,
            )
            nc.vector.tensor_tensor(
                out=k1[:, 1:NT], in0=k1[:, 1:NT], in1=validt[:, 1:NT],
                op=mybir.AluOpType.mult,
            )
            nc.vector.tensor_tensor(
                out=kp[:, 1:NT], in0=idxf[:, 1:NT], in1=idxf[:, 0 : NT - 1],
                op=mybir.AluOpType.not_equal,
            )
            nc.vector.tensor_tensor(
                out=keep_i[:, 1:NT], in0=kp[:, 1:NT], in1=k1[:, 1:NT],
                op=mybir.AluOpType.mult,
            )
            nc.vector.tensor_copy(out=preds_i[:, 0:1], in_=idxf[:, 0:1])
            nc.vector.tensor_scalar(
                out=k1[:, 0:1], in0=idxf[:, 0:1],
                scalar1=float(BLANK), scalar2=None,
                op0=mybir.AluOpType.not_equal,
            )
            nc.vector.tensor_tensor(
                out=k1[:, 0:1], in0=k1[:, 0:1], in1=validt[:, 0:1],
                op=mybir.AluOpType.mult,
            )
            nc.vector.tensor_tensor(
                out=kp[:, 0:1], in0=idxf[:, 0:1], in1=wrap[:],
                op=mybir.AluOpType.not_equal,
            )
            nc.vector.tensor_tensor(
                out=keep_i[:, 0:1], in0=kp[:, 0:1], in1=k1[:, 0:1],
                op=mybir.AluOpType.mult,
            )
            nc.sync.dma_start(out=preds_o[:], in_=preds_i[:])
            nc.sync.dma_start(out=keep_o[:], in_=keep_i[:])
    nc.compile()
    return nc


def _host_inputs(log_probs, input_lengths):
    log_probs = np.ascontiguousarray(np.asarray(log_probs, dtype=np.float32))
    lens = np.asarray(input_lengths).astype(np.int64)
    # valid[p, c] = ((p%32)*128 + c) < len(utterance p//32)
    tvals = (np.arange(P) % RPU)[:, None] * NT + np.arange(NT)[None, :]
    in_maps = []
    for core in range(NCORES):
        lp_c = log_probs[core * BLOC : (core + 1) * BLOC].reshape(F, V)
        lens_c = lens[core * BLOC : (core + 1) * BLOC]
        vmask = (tvals < lens_c[np.arange(P) // RPU][:, None]).astype(np.float32)
        in_maps.append({"lp": lp_c, "valid": np.ascontiguousarray(vmask)})
    return in_maps


def _grid_to_bt(arr):
    # arr [P, NT]: value for frame t = (p%32)*128 + c of utterance p//32
    return arr.reshape(BLOC, RPU * NT)


def kernel(log_probs, input_lengths, **_kw):
    if "nc" not in _CACHE:
        _CACHE["nc"] = _build_program()
    nc = _CACHE["nc"]
    in_maps = _host_inputs(log_probs, input_lengths)
    res = run_bass_kernel_spmd(nc, in_maps, core_ids=list(range(NCORES)))
    preds = np.empty((B, T), dtype=np.int32)
    keep = np.empty((B, T), dtype=bool)
    max_logp = np.empty((B, T), dtype=np.float32)
    for c, r in enumerate(res.results):
        sl = slice(c * BLOC, (c + 1) * BLOC)
        preds[sl] = _grid_to_bt(r["preds"])
        keep[sl] = _grid_to_bt(r["keep"]).astype(bool)
        max_logp[sl] = _grid_to_bt(r["maxlp"])
    return preds, keep, max_logp
